# revision 6
# baseline (speedup 1.0000x reference)
"""Trainium2 Bass kernel for a dense pre-LN transformer block.

Problem: B=2, T=2048, C=1024, H=16 heads, DH=64, FF=4096 (fp32).
  out = x + proj(attn(LN1(x))) + FFN(LN2(x + proj(...)))

Sharding (8 cores): sequence-parallel. Cores 0-3 own batch 0, cores 4-7 own
batch 1. Within a batch, core (local rank r) owns two 256-row blocks {r, 7-r}
of the 2048 sequence — causally balanced so every core does identical work.
k/v are computed locally and exchanged with ONE AllGather (bf16) inside each
4-core group; everything else is fully local (no all-reduce).

Precision: fp32 storage; matmuls in fp32r (full PE rate at free-dim>=256,
~1e-4 matmul error); attention internals (q,k,v,exp weights) in bf16
(~3e-3 attention error, verified against the fp32 reference offline).

Layout tricks:
  - LN + transpose fused: x-mean via tensor_scalar, then a matmul against
    diag(rstd) transposes the row-block AND applies the 1/std scale; gamma/beta
    applied per-partition post-transpose.
  - scores computed transposed (s on partitions, t on free) with 2 heads
    packed per matmul pair via tile_position row groups (K=64 each).
  - softmax denominator fused into attn@V as an extra all-ones lhsT column
    (M=65); normalization via reciprocal + K=1 broadcast-matmul + one
    DVE multiply, folded into building the proj lhsT.
  - causal mask = additive per-core mask tensors (host-generated data), so
    the SPMD program is identical on every core.
"""

import numpy as np
from contextlib import ExitStack

import concourse.bass as bass
import concourse.mybir as mybir
import concourse.tile as tile
from concourse import bacc
from concourse.bass_utils import run_bass_kernel_spmd

P = 128
B, T, C = 2, 2048, 1024
H, DH = 16, 64
FF = 4 * C
NCORES = 8
RPC = 512            # rows per core
NTB = 4              # 128-row t-blocks per core
NPAIR = 8            # head pairs
KT_ELEMS = C * RPC   # kT contribution elems per core
VT_ELEMS = RPC * C
CHUNK = KT_ELEMS + VT_ELEMS
MASK_NEG = -30000.0

f32 = mybir.dt.float32
f32r = mybir.dt.float32r
bf16 = mybir.dt.bfloat16
AF = mybir.ActivationFunctionType


def _t128_list(r):
    """Global 128-row block indices (within the batch) owned by local rank r,
    in local row order."""
    return [2 * r, 2 * r + 1, 2 * (7 - r), 2 * (7 - r) + 1]


def build_nc():
    nc = bacc.Bacc(num_devices=NCORES)

    xr = nc.dram_tensor("xr", [RPC, C], f32, kind="ExternalInput")
    wq = nc.dram_tensor("wq", [C, C], f32, kind="ExternalInput")
    wk = nc.dram_tensor("wk", [C, C], f32, kind="ExternalInput")
    wv = nc.dram_tensor("wv", [C, C], f32, kind="ExternalInput")
    wp = nc.dram_tensor("wp", [C, C], f32, kind="ExternalInput")
    bp = nc.dram_tensor("bp", [C], f32, kind="ExternalInput")
    w1 = nc.dram_tensor("w1", [C, FF], f32, kind="ExternalInput")
    b1 = nc.dram_tensor("b1", [FF], f32, kind="ExternalInput")
    w2 = nc.dram_tensor("w2", [FF, C], f32, kind="ExternalInput")
    b2 = nc.dram_tensor("b2", [C], f32, kind="ExternalInput")
    g1 = nc.dram_tensor("g1", [C], f32, kind="ExternalInput")
    be1 = nc.dram_tensor("be1", [C], f32, kind="ExternalInput")
    g2 = nc.dram_tensor("g2", [C], f32, kind="ExternalInput")
    be2 = nc.dram_tensor("be2", [C], f32, kind="ExternalInput")
    ident = nc.dram_tensor("ident", [P, P], f32, kind="ExternalInput")
    onesd = nc.dram_tensor("onesd", [1, DH], f32, kind="ExternalInput")
    masks = nc.dram_tensor("masks", [8, 2, P, RPC], f32, kind="ExternalInput")
    outr = nc.dram_tensor("outr", [RPC, C], f32, kind="ExternalOutput")

    kv_in = nc.dram_tensor("kv_in", [CHUNK], bf16)
    kv_out = nc.dram_tensor("kv_out", [4, CHUNK], bf16)

    with tile.TileContext(nc) as tc, \
         nc.allow_low_precision(reason="fp32r matmuls + bf16 attention by design"), \
         ExitStack() as ctx:
        const = ctx.enter_context(tc.tile_pool(name="const", bufs=1))
        res = ctx.enter_context(tc.tile_pool(name="res", bufs=1))

        # ---- constants ----
        id_sb = const.tile([P, P], f32)
        nc.sync.dma_start(out=id_sb, in_=ident[:, :])
        ones64 = const.tile([1, DH], f32r)
        nc.sync.dma_start(out=ones64, in_=onesd[:, :].bitcast(f32r))
        eps_sb = const.tile([P, 1], f32)
        nc.vector.memset(eps_sb, 1e-5)
        g1_sb = const.tile([P, 8], f32)
        nc.sync.dma_start(out=g1_sb, in_=g1.ap().rearrange("(k p) -> p k", p=P))
        be1_sb = const.tile([P, 8], f32)
        nc.sync.dma_start(out=be1_sb, in_=be1.ap().rearrange("(k p) -> p k", p=P))
        g2_sb = const.tile([P, 8], f32)
        nc.sync.dma_start(out=g2_sb, in_=g2.ap().rearrange("(k p) -> p k", p=P))
        be2_sb = const.tile([P, 8], f32)
        nc.sync.dma_start(out=be2_sb, in_=be2.ap().rearrange("(k p) -> p k", p=P))
        b1_sb = const.tile([P, 32], f32)
        nc.sync.dma_start(out=b1_sb, in_=b1.ap().rearrange("(k p) -> p k", p=P))
        bp_bc = const.tile([P, C], f32)
        nc.sync.dma_start(
            out=bp_bc, in_=bass.AP(tensor=bp, offset=0, ap=[[0, P], [1, C]]))
        b2_bc = const.tile([P, C], f32)
        nc.sync.dma_start(
            out=b2_bc, in_=bass.AP(tensor=b2, offset=0, ap=[[0, P], [1, C]]))

        # ---- residents ----
        x_sb = res.tile([P, NTB, C], f32)            # x rows; becomes x2 in place
        qT_sb = res.tile([P, NPAIR, RPC], bf16)      # qT, pair-pblocks
        anorm = res.tile([P, NPAIR, RPC], f32r)      # normalized attnT (proj lhsT)

        nc.sync.dma_start(
            out=x_sb, in_=xr[:, :].rearrange("(tb p) c -> p tb c", p=P))

        def layer_norm_transpose(src_view, g_sb, be_sb, dst, pool, pspool):
            """src_view: [P, NTB, C] f32 rows; dst [P, 8, RPC] f32r = transposed
            normalized (gamma/beta applied)."""
            for tb in range(NTB):
                stats = pool.tile([P, 2, 6], f32, tag="ln_stats")
                for i in range(2):
                    nc.vector.bn_stats(out=stats[:, i, :],
                                       in_=src_view[:, tb, i * 512:(i + 1) * 512])
                mv = pool.tile([P, 2], f32, tag="ln_mv")
                nc.vector.bn_aggr(out=mv, in_=stats)
                rstd = pool.tile([P, 1], f32, tag="ln_rstd")
                nc.scalar.activation(out=rstd, in_=mv[:, 1:2], func=AF.Sqrt,
                                     bias=eps_sb)
                nc.vector.reciprocal(out=rstd, in_=rstd)
                xc = pool.tile([P, C], f32, tag="ln_xc")
                nc.vector.tensor_scalar(out=xc, in0=src_view[:, tb, :],
                                        scalar1=mv[:, 0:1], scalar2=None,
                                        op0=mybir.AluOpType.subtract)
                diag = pool.tile([P, P], f32, tag="ln_diag")
                nc.vector.tensor_scalar_mul(out=diag, in0=id_sb, scalar1=rstd)
                for cc in range(8):
                    pt = pspool.tile([P, P], f32, tag="ln_tp")
                    nc.tensor.matmul(pt, xc[:, cc * P:(cc + 1) * P], diag[:, :],
                                     start=True, stop=True)
                    nc.vector.tensor_scalar(
                        out=dst[:, cc, tb * P:(tb + 1) * P], in0=pt,
                        scalar1=g_sb[:, cc:cc + 1], scalar2=be_sb[:, cc:cc + 1],
                        op0=mybir.AluOpType.mult, op1=mybir.AluOpType.add)

        # ================= phase 1+2: LN1, qkv =================
        with tc.tile_pool(name="ph12", bufs=1) as pA, \
             tc.tile_pool(name="ph12s", bufs=2) as pAs, \
             tc.tile_pool(name="ps12b", bufs=3, space="PSUM") as psA2:
            hT = pA.tile([P, 8, RPC], f32r)
            with nc.named_scope("ln1"):
                layer_norm_transpose(x_sb, g1_sb, be1_sb, hT, pAs, psA2)

            kT_own = pA.tile([P, 8, RPC], bf16)
            v_own = pA.tile([P, NTB, C], bf16)

            with nc.named_scope("qkv"):
                # qT / kT: out [d-pblock 128, t 512]
                for w_dram, dst in ((wq, qT_sb), (wk, kT_own)):
                    for pb in range(8):
                        wl = pAs.tile([P, 8, P], f32r, tag="wl")
                        nc.sync.dma_start(
                            out=wl,
                            in_=w_dram[:, pb * P:(pb + 1) * P]
                            .rearrange("(cc p) d -> p cc d", p=P).bitcast(f32r))
                        pq = psA2.tile([P, RPC], f32, tag="pq")
                        for cc in range(8):
                            nc.tensor.matmul(pq, wl[:, cc, :], hT[:, cc, :],
                                             start=(cc == 0), stop=(cc == 7))
                        nc.scalar.copy(out=dst[:, pb, :], in_=pq)
                # v: out [t-block 128, hd 1024]
                wv_sb = pA.tile([P, 8, C], f32r)
                nc.sync.dma_start(
                    out=wv_sb,
                    in_=wv[:, :].rearrange("(cc p) d -> p cc d", p=P).bitcast(f32r))
                for tb in range(NTB):
                    for nn in range(2):
                        pv = psA2.tile([P, 512], f32, tag="pq")
                        for cc in range(8):
                            nc.tensor.matmul(
                                pv, hT[:, cc, tb * P:(tb + 1) * P],
                                wv_sb[:, cc, nn * 512:(nn + 1) * 512],
                                start=(cc == 0), stop=(cc == 7))
                        nc.scalar.copy(out=v_own[:, tb, nn * 512:(nn + 1) * 512],
                                       in_=pv)

                nc.sync.dma_start(
                    out=kv_in.ap()[0:KT_ELEMS]
                    .rearrange("(pb p t) -> p pb t", p=P, t=RPC),
                    in_=kT_own)
                nc.sync.dma_start(
                    out=kv_in.ap()[KT_ELEMS:CHUNK]
                    .rearrange("(tb p c) -> p tb c", p=P, c=C),
                    in_=v_own)

        # ================= phase 3: AllGather k/v =================
        with nc.named_scope("allgather"):
            nc.gpsimd.collective_compute(
                "AllGather", mybir.AluOpType.bypass,
                ins=[kv_in[:]], outs=[kv_out[:]],
                replica_groups=[[0, 1, 2, 3], [4, 5, 6, 7]],
            )

        # ================= phase 4: attention =================
        with tc.tile_pool(name="attn", bufs=1) as pB, \
             tc.tile_pool(name="attns", bufs=4) as pBs, \
             tc.tile_pool(name="attns2", bufs=2) as pBs2, \
             tc.tile_pool(name="ps_sc", bufs=4, space="PSUM") as ps_sc, \
             tc.tile_pool(name="ps_at", bufs=1, space="PSUM") as ps_at:
            kT_cache = pB.tile([P, 8, T], bf16)
            v_cache = pB.tile([P, 16, H, DH + 1], bf16)
            mask_cache = pB.tile([P, 8, 2, RPC], f32)

            with nc.named_scope("kv_load"):
                nc.sync.dma_start(
                    out=mask_cache,
                    in_=masks[:, :, :, :].rearrange("s m i j -> i s m j"))
                for rho in range(4):
                    ktv = kv_out[rho, 0:KT_ELEMS].rearrange(
                        "(pb p t) -> p pb t", p=P, t=RPC)
                    for eta in range(2):
                        s256 = rho if eta == 0 else 7 - rho
                        nc.sync.dma_start(
                            out=kT_cache[:, :, s256 * 256:(s256 + 1) * 256],
                            in_=ktv[:, :, eta * 256:(eta + 1) * 256])
                    vv = kv_out[rho, KT_ELEMS:CHUNK].rearrange(
                        "(tb p c) -> p tb c", p=P, c=C)
                    for ell in range(4):
                        s256 = rho if ell < 2 else 7 - rho
                        sm = s256 * 2 + (ell % 2)
                        nc.sync.dma_start(
                            out=v_cache[:, sm, :, 0:DH],
                            in_=vv[:, ell, :].rearrange("p (h d) -> p h d", h=H))
                nc.gpsimd.memset(v_cache[:, :, :, DH], 1.0)

            with nc.named_scope("attention"):
                for pr in range(NPAIR):
                    pa = ps_at.tile([DH + 1, RPC], f32, tag="attnA")
                    pb_ = ps_at.tile([DH + 1, RPC], f32, tag="attnB")
                    for s256 in range(8):
                        for m in range(2):
                            sm = 2 * s256 + m
                            sA = ps_sc.tile([P, RPC], f32, tag="sc")
                            sB = ps_sc.tile([P, RPC], f32, tag="sc")
                            nc.tensor.matmul(
                                sA, kT_cache[0:DH, pr, sm * P:(sm + 1) * P],
                                qT_sb[0:DH, pr, :], start=True, stop=True,
                                tile_position=(0, 0))
                            nc.tensor.matmul(
                                sB, kT_cache[DH:P, pr, sm * P:(sm + 1) * P],
                                qT_sb[DH:P, pr, :], start=True, stop=True,
                                tile_position=(64, 0))
                            mk = mask_cache[:, s256, m, :]
                            nc.vector.tensor_add(out=sA, in0=sA, in1=mk)
                            nc.vector.tensor_add(out=sB, in0=sB, in1=mk)
                            eA = pBs.tile([P, RPC], bf16, tag="eA")
                            eB = pBs.tile([P, RPC], bf16, tag="eB")
                            nc.scalar.activation(out=eA, in_=sA, func=AF.Exp)
                            nc.scalar.activation(out=eB, in_=sB, func=AF.Exp)
                            nc.tensor.matmul(
                                pa, v_cache[:, sm, 2 * pr, :], eA[:, :],
                                start=(sm == 0), stop=(sm == 15))
                            nc.tensor.matmul(
                                pb_, v_cache[:, sm, 2 * pr + 1, :], eB[:, :],
                                start=(sm == 0), stop=(sm == 15))
                    # normalize: rows 0:64 / row 64 (the ones-column sums)
                    for head_ps, half in ((pa, 0), (pb_, 1)):
                        inv = pBs2.tile([1, RPC], f32r, tag="inv")
                        nc.vector.reciprocal(
                            out=inv, in_=head_ps[DH:DH + 1, :].bitcast(f32r))
                        bc = ps_sc.tile([DH, RPC], f32, tag="bc", bufs=2)
                        nc.tensor.matmul(bc, ones64[:, :], inv[:, :],
                                         start=True, stop=True)
                        bcs = pBs2.tile([DH, RPC], f32, tag="bcs")
                        nc.scalar.copy(out=bcs, in_=bc)
                        nc.vector.tensor_mul(
                            out=anorm[half * DH:(half + 1) * DH, pr, :],
                            in0=head_ps[0:DH, :], in1=bcs)

        # ================= phase 5: proj + residual + LN2 =================
        with tc.tile_pool(name="ffn", bufs=1) as pC, \
             tc.tile_pool(name="ffns", bufs=3) as pCs, \
             tc.tile_pool(name="ffns2", bufs=2) as pCs2, \
             tc.tile_pool(name="ps_p", bufs=2, space="PSUM") as ps_p:
            with nc.named_scope("proj"):
                for nn in range(2):
                    pps = [ps_p.tile([P, 512], f32, tag=f"po{tb}", bufs=1, name=f"pp{tb}")
                           for tb in range(NTB)]
                    for pr in range(NPAIR):
                        wpl = pCs.tile([P, 512], f32r, tag="wpl")
                        nc.sync.dma_start(
                            out=wpl,
                            in_=wp[pr * P:(pr + 1) * P,
                                   nn * 512:(nn + 1) * 512].bitcast(f32r))
                        for tb in range(NTB):
                            nc.tensor.matmul(
                                pps[tb], anorm[:, pr, tb * P:(tb + 1) * P],
                                wpl[:, :], start=(pr == 0), stop=(pr == 7))
                    for tb in range(NTB):
                        sl = x_sb[:, tb, nn * 512:(nn + 1) * 512]
                        nc.vector.tensor_add(out=sl, in0=sl, in1=pps[tb])
                        nc.vector.tensor_add(
                            out=sl, in0=sl, in1=bp_bc[:, nn * 512:(nn + 1) * 512])

            h2T = pC.tile([P, 8, RPC], f32r)
            with nc.named_scope("ln2"):
                layer_norm_transpose(x_sb, g2_sb, be2_sb, h2T, pCs2, ps_p)

            # ================= phase 6: FFN =================
            ffT = pC.tile([P, 32, RPC], f32r)
            with nc.named_scope("ffn1"):
                for fb in range(32):
                    w1l = pCs.tile([P, 8, P], f32r, tag="w1l")
                    nc.sync.dma_start(
                        out=w1l,
                        in_=w1[:, fb * P:(fb + 1) * P]
                        .rearrange("(cc p) f -> p cc f", p=P).bitcast(f32r))
                    pf = ps_p.tile([P, RPC], f32, tag="pf")
                    for cc in range(8):
                        nc.tensor.matmul(pf, w1l[:, cc, :], h2T[:, cc, :],
                                         start=(cc == 0), stop=(cc == 7))
                    nc.scalar.activation(out=ffT[:, fb, :], in_=pf, func=AF.Relu,
                                         bias=b1_sb[:, fb:fb + 1])
            with nc.named_scope("ffn2"):
                for nn in range(2):
                    pos = [ps_p.tile([P, 512], f32, tag=f"po{tb}", bufs=1, name=f"po{tb}")
                           for tb in range(NTB)]
                    for fb in range(32):
                        w2l = pCs.tile([P, 512], f32r, tag="w2l")
                        nc.sync.dma_start(
                            out=w2l,
                            in_=w2[fb * P:(fb + 1) * P,
                                   nn * 512:(nn + 1) * 512].bitcast(f32r))
                        for tb in range(NTB):
                            nc.tensor.matmul(
                                pos[tb], ffT[:, fb, tb * P:(tb + 1) * P],
                                w2l[:, :], start=(fb == 0), stop=(fb == 31))
                    for tb in range(NTB):
                        ot = pCs2.tile([P, 512], f32, tag="ot")
                        nc.vector.tensor_add(
                            out=ot, in0=pos[tb],
                            in1=x_sb[:, tb, nn * 512:(nn + 1) * 512])
                        nc.vector.tensor_add(
                            out=ot, in0=ot, in1=b2_bc[:, nn * 512:(nn + 1) * 512])
                        nc.sync.dma_start(
                            out=outr[tb * P:(tb + 1) * P,
                                     nn * 512:(nn + 1) * 512],
                            in_=ot)

    nc.finalize()
    return nc


_NC = None
LAST_RESULTS = None


def _get_nc():
    global _NC
    if _NC is None:
        _NC = build_nc()
    return _NC


def _make_masks(r):
    """Additive causal masks for local rank r: [8, 2, 128, 512] f32."""
    t128 = _t128_list(r)
    tglob = np.concatenate([tb * P + np.arange(P) for tb in t128])  # [512]
    mk = np.zeros((8, 2, P, RPC), np.float32)
    for s256 in range(8):
        for m in range(2):
            sglob = (2 * s256 + m) * P + np.arange(P)  # [128]
            mk[s256, m] = np.where(sglob[:, None] <= tglob[None, :],
                                   0.0, MASK_NEG)
    return mk


def kernel(**inputs):
    x = np.ascontiguousarray(np.asarray(inputs["x"], dtype=np.float32))
    wq = np.asarray(inputs["wq"], dtype=np.float32)   # [H, C, DH]
    wk = np.asarray(inputs["wk"], dtype=np.float32)
    wv = np.asarray(inputs["wv"], dtype=np.float32)

    scale = DH ** -0.5
    # flatten per-head weights: [C, H*DH]; fold the score scale into wq
    wq_flat = np.ascontiguousarray(
        (wq * scale).transpose(1, 0, 2).reshape(C, H * DH).astype(np.float32))
    wk_flat = np.ascontiguousarray(
        wk.transpose(1, 0, 2).reshape(C, H * DH).astype(np.float32))
    wv_flat = np.ascontiguousarray(
        wv.transpose(1, 0, 2).reshape(C, H * DH).astype(np.float32))

    common = {
        "wq": wq_flat,
        "wk": wk_flat,
        "wv": wv_flat,
        "wp": np.ascontiguousarray(np.asarray(inputs["w_proj"], np.float32)),
        "bp": np.ascontiguousarray(np.asarray(inputs["b_proj"], np.float32)),
        "w1": np.ascontiguousarray(np.asarray(inputs["w1"], np.float32)),
        "b1": np.ascontiguousarray(np.asarray(inputs["b1"], np.float32)),
        "w2": np.ascontiguousarray(np.asarray(inputs["w2"], np.float32)),
        "b2": np.ascontiguousarray(np.asarray(inputs["b2"], np.float32)),
        "g1": np.ascontiguousarray(np.asarray(inputs["g1"], np.float32)),
        "be1": np.ascontiguousarray(np.asarray(inputs["be1"], np.float32)),
        "g2": np.ascontiguousarray(np.asarray(inputs["g2"], np.float32)),
        "be2": np.ascontiguousarray(np.asarray(inputs["be2"], np.float32)),
        "ident": np.eye(P, dtype=np.float32),
        "onesd": np.ones((1, DH), dtype=np.float32),
    }

    in_maps = []
    for c in range(NCORES):
        g, r = c // 4, c % 4
        rows = np.concatenate(
            [x[g, 256 * r:256 * r + 256], x[g, 256 * (7 - r):256 * (7 - r) + 256]])
        m = dict(common)
        m["xr"] = np.ascontiguousarray(rows)
        m["masks"] = _make_masks(r)
        in_maps.append(m)

    nc = _get_nc()
    res = run_bass_kernel_spmd(nc, in_maps, core_ids=list(range(NCORES)))
    global LAST_RESULTS
    LAST_RESULTS = res

    out = np.empty((B, T, C), np.float32)
    for c in range(NCORES):
        g, r = c // 4, c % 4
        o = res.results[c]["outr"]
        out[g, 256 * r:256 * r + 256] = o[0:256]
        out[g, 256 * (7 - r):256 * (7 - r) + 256] = o[256:512]
    return out


# revision 17
# speedup vs baseline: 1.3224x; 1.3224x over previous
"""Trainium2 Bass kernel for a dense pre-LN transformer block.

Problem: B=2, T=2048, C=1024, H=16 heads, DH=64, FF=4096 (fp32).
  out = x + proj(attn(LN1(x))) + FFN(LN2(x + proj(...)))

Sharding (8 cores): sequence-parallel. Cores 0-3 own batch 0, cores 4-7 own
batch 1. Within a batch, core (local rank r) owns two 256-row blocks {r, 7-r}
of the 2048 sequence — causally balanced so every core does identical work.
k/v are computed locally and exchanged with TWO AllGathers (bf16, k first so
score matmuls can start while v is still in flight) inside each 4-core group;
everything else is fully local (no all-reduce).

Precision: fp32 storage; matmuls in fp32r (full PE rate at free-dim>=256,
~1e-4 matmul error); attention internals (q,k,v,exp weights) in bf16
(~3e-3 attention error, verified against the fp32 reference offline).

Structure highlights:
  - LN + transpose fused via matmul against diag(rstd).
  - scores computed transposed (s on partitions, t on free), 2 heads packed
    per matmul pair via tile_position row groups (K=64 each), both heads'
    scores land in ONE 2-bank psum tile so a single ACT exp covers both.
  - trapezoid causal: s-blocks 0..7 score against all 512 t-cols, s-blocks
    8..15 only against the late 256 t-cols (uniform across cores; residual
    causal masking is multiplicative bf16 0/1 data applied post-exp on DVE).
  - softmax denominator fused into attn@V as an extra all-ones lhsT column
    (M=65); normalization via fast reciprocal + K=1 broadcast-matmul.
"""

import numpy as np
import ml_dtypes
from contextlib import ExitStack

import concourse.bass as bass
import concourse.mybir as mybir
import concourse.tile as tile
from concourse import bacc
from concourse.bass_utils import run_bass_kernel_spmd
from concourse.tile_rust import add_dep_helper

P = 128
B, T, C = 2, 2048, 1024
H, DH = 16, 64
FF = 4 * C
NCORES = 8
RPC = 512            # rows per core
NTB = 4              # 128-row t-blocks per core
NPAIR = 8            # head pairs
KT_ELEMS = C * RPC
VT_ELEMS = RPC * C

f32 = mybir.dt.float32
f32r = mybir.dt.float32r
bf16 = mybir.dt.bfloat16
AF = mybir.ActivationFunctionType


def _t128_list(r):
    return [2 * r, 2 * r + 1, 2 * (7 - r), 2 * (7 - r) + 1]


def build_nc():
    nc = bacc.Bacc(num_devices=NCORES)

    xr = nc.dram_tensor("xr", [RPC, C], f32, kind="ExternalInput")
    wq = nc.dram_tensor("wq", [C, C], f32, kind="ExternalInput")
    wk = nc.dram_tensor("wk", [C, C], f32, kind="ExternalInput")
    wv = nc.dram_tensor("wv", [C, C], f32, kind="ExternalInput")
    wp = nc.dram_tensor("wp", [C, C], f32, kind="ExternalInput")
    bp = nc.dram_tensor("bp", [C], f32, kind="ExternalInput")
    w1 = nc.dram_tensor("w1", [C, FF], f32, kind="ExternalInput")
    b1 = nc.dram_tensor("b1", [FF], f32, kind="ExternalInput")
    w2 = nc.dram_tensor("w2", [FF, C], bf16, kind="ExternalInput")
    b2 = nc.dram_tensor("b2", [C], f32, kind="ExternalInput")
    g1 = nc.dram_tensor("g1", [C], f32, kind="ExternalInput")
    be1 = nc.dram_tensor("be1", [C], f32, kind="ExternalInput")
    g2 = nc.dram_tensor("g2", [C], f32, kind="ExternalInput")
    be2 = nc.dram_tensor("be2", [C], f32, kind="ExternalInput")
    ident = nc.dram_tensor("ident", [P, P], f32, kind="ExternalInput")
    onesd = nc.dram_tensor("onesd", [1, DH], f32, kind="ExternalInput")
    # multiplicative causal masks (1 keep / 0 drop), both heads' column
    # ranges duplicated side by side:
    mask_lo = nc.dram_tensor("mask_lo", [4, 2, P, 2 * RPC], bf16,
                             kind="ExternalInput")
    mask_hi = nc.dram_tensor("mask_hi", [4, 2, P, RPC], bf16,
                             kind="ExternalInput")
    outr = nc.dram_tensor("outr", [RPC, C], f32, kind="ExternalOutput")


    with tile.TileContext(nc) as tc, \
         nc.allow_low_precision(reason="fp32r matmuls + bf16 attention by design"), \
         ExitStack() as top:
        const = top.enter_context(tc.tile_pool(name="const", bufs=1))
        res = top.enter_context(tc.tile_pool(name="res", bufs=1))

        # ---- constants ----
        id_sb = const.tile([P, P], f32)
        nc.sync.dma_start(out=id_sb, in_=ident[:, :])
        ones64 = const.tile([1, DH], f32)
        nc.sync.dma_start(out=ones64, in_=onesd[:, :])
        eps_sb = const.tile([P, 1], f32)
        nc.vector.memset(eps_sb, 1e-5)
        g1_sb = const.tile([P, 8], f32)
        nc.sync.dma_start(out=g1_sb, in_=g1.ap().rearrange("(k p) -> p k", p=P))
        be1_sb = const.tile([P, 8], f32)
        nc.sync.dma_start(out=be1_sb, in_=be1.ap().rearrange("(k p) -> p k", p=P))
        g2_sb = const.tile([P, 8], f32)
        nc.sync.dma_start(out=g2_sb, in_=g2.ap().rearrange("(k p) -> p k", p=P))
        be2_sb = const.tile([P, 8], f32)
        nc.sync.dma_start(out=be2_sb, in_=be2.ap().rearrange("(k p) -> p k", p=P))
        b1_sb = const.tile([P, 32], f32)
        nc.sync.dma_start(out=b1_sb, in_=b1.ap().rearrange("(k p) -> p k", p=P))
        bp_bc = const.tile([P, C], f32)
        nc.sync.dma_start(
            out=bp_bc, in_=bass.AP(tensor=bp, offset=0, ap=[[0, P], [1, C]]))
        b2_bc = const.tile([P, C], f32)
        nc.sync.dma_start(
            out=b2_bc, in_=bass.AP(tensor=b2, offset=0, ap=[[0, P], [1, C]]))

        # ---- whole-kernel residents ----
        x_sb = res.tile([P, NTB, C], f32)        # x rows; becomes x2 in place
        nc.sync.dma_start(
            out=x_sb, in_=xr[:, :].rearrange("(tb p) c -> p tb c", p=P))

        def layer_norm_transpose(src_view, g_sb, be_sb, dst, pool, pspool):
            for tb in range(NTB):
                stats = pool.tile([P, 2, 6], f32, tag="ln_stats")
                for i in range(2):
                    nc.vector.bn_stats(out=stats[:, i, :],
                                       in_=src_view[:, tb, i * 512:(i + 1) * 512])
                mv = pool.tile([P, 2], f32, tag="ln_mv")
                nc.vector.bn_aggr(out=mv, in_=stats)
                rstd = pool.tile([P, 1], f32, tag="ln_rstd")
                nc.scalar.activation(out=rstd, in_=mv[:, 1:2], func=AF.Sqrt,
                                     bias=eps_sb)
                nc.vector.reciprocal(out=rstd, in_=rstd)
                xc = pool.tile([P, C], f32, tag="ln_xc")
                nc.vector.tensor_scalar(out=xc, in0=src_view[:, tb, :],
                                        scalar1=mv[:, 0:1], scalar2=None,
                                        op0=mybir.AluOpType.subtract)
                diag = pool.tile([P, P], f32, tag="ln_diag")
                nc.vector.tensor_scalar_mul(out=diag, in0=id_sb, scalar1=rstd)
                for cc in range(8):
                    pt = pspool.tile([P, P], f32, tag="ln_tp")
                    nc.tensor.matmul(pt, xc[:, cc * P:(cc + 1) * P], diag[:, :],
                                     start=True, stop=True)
                    nc.vector.tensor_scalar(
                        out=dst[:, cc, tb * P:(tb + 1) * P], in0=pt,
                        scalar1=g_sb[:, cc:cc + 1], scalar2=be_sb[:, cc:cc + 1],
                        op0=mybir.AluOpType.mult, op1=mybir.AluOpType.add)

        dram = top.enter_context(tc.tile_pool(name="dram", bufs=1, space="DRAM"))
        k_in = dram.tile([P, 8, RPC], bf16)
        k_out = dram.tile([4, P, 8, RPC], bf16)
        v_in = dram.tile([P, NTB, C], bf16)
        v_out = dram.tile([4, P, NTB, C], bf16)

        qa = top.enter_context(tc.tile_pool(name="qa", bufs=1))
        qT_sb = qa.tile([P, NPAIR, RPC], bf16)
        anorm = qa.tile([P, NPAIR, RPC], f32r)

        # ========== phase 1+2: LN1, k -> AG1, v -> AG2, q ==========
        with tc.tile_pool(name="ph12", bufs=1) as pA, \
             tc.tile_pool(name="ph12s", bufs=2) as pAs, \
             tc.tile_pool(name="ps12", bufs=3, space="PSUM") as psA:
            hT = pA.tile([P, 8, RPC], f32r)
            layer_norm_transpose(x_sb, g1_sb, be1_sb, hT, pAs, psA)

            wk_sb = pAs.tile([P, 8, C], f32r, tag="wbig")
            nc.sync.dma_start(
                out=wk_sb,
                in_=wk[:, :].rearrange("(cc p) d -> p cc d", p=P).bitcast(f32r))
            kT_own = pA.tile([P, 8, RPC], bf16)
            for pb in range(8):
                pq = psA.tile([P, RPC], f32, tag="pq")
                for cc in range(8):
                    nc.tensor.matmul(pq, wk_sb[:, cc, pb * P:(pb + 1) * P],
                                     hT[:, cc, :], start=(cc == 0), stop=(cc == 7))
                nc.scalar.copy(out=kT_own[:, pb, :], in_=pq)
            nc.sync.dma_start(out=k_in[:], in_=kT_own)
            cc_k = nc.gpsimd.collective_compute(
                "AllGather", mybir.AluOpType.bypass,
                ins=[k_in.opt()], outs=[k_out.opt()],
                replica_groups=[[0, 1, 2, 3], [4, 5, 6, 7]])

            wv_sb = pAs.tile([P, 8, C], f32r, tag="wbig")
            nc.sync.dma_start(
                out=wv_sb,
                in_=wv[:, :].rearrange("(cc p) d -> p cc d", p=P).bitcast(f32r))
            v_own = pA.tile([P, NTB, C], bf16)
            for tb in range(NTB):
                for nn in range(2):
                    pv = psA.tile([P, 512], f32, tag="pq")
                    for cc in range(8):
                        nc.tensor.matmul(
                            pv, hT[:, cc, tb * P:(tb + 1) * P],
                            wv_sb[:, cc, nn * 512:(nn + 1) * 512],
                            start=(cc == 0), stop=(cc == 7))
                    nc.scalar.copy(out=v_own[:, tb, nn * 512:(nn + 1) * 512],
                                   in_=pv)
            nc.sync.dma_start(out=v_in[:], in_=v_own)
            cc_v = nc.gpsimd.collective_compute(
                "AllGather", mybir.AluOpType.bypass,
                ins=[v_in.opt()], outs=[v_out.opt()],
                replica_groups=[[0, 1, 2, 3], [4, 5, 6, 7]])
            add_dep_helper(cc_v.ins, cc_k.ins, sync=True,
                           reason="serialize AG2 after AG1 (ncfw)")

            wq_sb = pAs.tile([P, 8, C], f32r, tag="wbig")
            nc.sync.dma_start(
                out=wq_sb,
                in_=wq[:, :].rearrange("(cc p) d -> p cc d", p=P).bitcast(f32r))
            for pb in range(8):
                pq = psA.tile([P, RPC], f32, tag="pq")
                for cc in range(8):
                    nc.tensor.matmul(pq, wq_sb[:, cc, pb * P:(pb + 1) * P],
                                     hT[:, cc, :], start=(cc == 0), stop=(cc == 7))
                nc.scalar.copy(out=qT_sb[:, pb, :], in_=pq)

        # ========== phase 4: attention ==========
        with tc.tile_pool(name="attn", bufs=1) as pB, \
             tc.tile_pool(name="attne", bufs=6) as pBe, \
             tc.tile_pool(name="attns", bufs=2) as pBs, \
             tc.tile_pool(name="ps_sc", bufs=2, space="PSUM") as ps_sc, \
             tc.tile_pool(name="ps_at", bufs=1, space="PSUM") as ps_at:
            kT_cache = pB.tile([P, 8, T], bf16)
            for rho in range(4):
                ktv = k_out[rho]
                for eta in range(2):
                    s256 = rho if eta == 0 else 7 - rho
                    dk = nc.gpsimd.dma_start(
                        out=kT_cache[:, :, s256 * 256:(s256 + 1) * 256],
                        in_=ktv[:, :, eta * 256:(eta + 1) * 256])
                    add_dep_helper(dk.ins, cc_k.ins, sync=True,
                                   reason="kT cache reads AG1 output")
            ml_sb = pB.tile([P, 4, 2, 2 * RPC], bf16)
            nc.gpsimd.dma_start(
                out=ml_sb, in_=mask_lo[:, :, :, :].rearrange("s m i j -> i s m j"))
            mh_sb = pB.tile([P, 4, 2, RPC], bf16)
            nc.gpsimd.dma_start(
                out=mh_sb, in_=mask_hi[:, :, :, :].rearrange("s m i j -> i s m j"))
            v_cache = pB.tile([P, 16, H, DH + 1], bf16)
            for rho in range(4):
                vv = v_out[rho]
                for ell in range(4):
                    s256 = rho if ell < 2 else 7 - rho
                    sm = s256 * 2 + (ell % 2)
                    dv = nc.gpsimd.dma_start(
                        out=v_cache[:, sm, :, 0:DH],
                        in_=vv[:, ell, :].rearrange("p (h d) -> p h d", h=H))
                    add_dep_helper(dv.ins, cc_v.ins, sync=True,
                                   reason="v cache reads AG2 output")
            nc.vector.memset(v_cache[:, :, :, DH], 1.0)

            for pr in range(NPAIR):
                pa = ps_at.tile([DH + 1, RPC], f32, tag="attnA")
                pb_ = ps_at.tile([DH + 1, RPC], f32, tag="attnB")
                for sm in range(16):
                    s256, m = sm // 2, sm % 2
                    if sm < 8:
                        sc = ps_sc.tile([P, 2 * RPC], f32, tag="sc")
                        nc.tensor.matmul(
                            sc[:, 0:RPC], kT_cache[0:DH, pr, sm * P:(sm + 1) * P],
                            qT_sb[0:DH, pr, :], start=True, stop=True,
                            tile_position=(0, 0))
                        nc.tensor.matmul(
                            sc[:, RPC:2 * RPC],
                            kT_cache[DH:P, pr, sm * P:(sm + 1) * P],
                            qT_sb[DH:P, pr, :], start=True, stop=True,
                            tile_position=(64, 0))
                        e = pBe.tile([P, 2 * RPC], bf16, tag="e")
                        nc.scalar.activation(out=e, in_=sc, func=AF.Exp)
                        nc.vector.tensor_mul(out=e, in0=e,
                                             in1=ml_sb[:, s256, m, :])
                        # full-width: start=True on sm==0 clears the whole
                        # bank; everything after accumulates
                        nc.tensor.matmul(
                            pa, v_cache[:, sm, 2 * pr, :], e[:, 0:RPC],
                            start=(sm == 0), stop=False, skip_group_check=True)
                        nc.tensor.matmul(
                            pb_, v_cache[:, sm, 2 * pr + 1, :], e[:, RPC:2 * RPC],
                            start=(sm == 0), stop=False, skip_group_check=True)
                    else:
                        sc = ps_sc.tile([P, 2 * RPC], f32, tag="sc")
                        # A and B in different PSUM banks (concurrent packed
                        # matmuls must not write the same bank)
                        nc.tensor.matmul(
                            sc[:, 0:256], kT_cache[0:DH, pr, sm * P:(sm + 1) * P],
                            qT_sb[0:DH, pr, 256:RPC], start=True, stop=True,
                            tile_position=(0, 0))
                        nc.tensor.matmul(
                            sc[:, RPC:RPC + 256],
                            kT_cache[DH:P, pr, sm * P:(sm + 1) * P],
                            qT_sb[DH:P, pr, 256:RPC], start=True, stop=True,
                            tile_position=(64, 0))
                        e = pBe.tile([P, RPC], bf16, tag="eh")
                        nc.scalar.activation(out=e[:, 0:256], in_=sc[:, 0:256],
                                             func=AF.Exp)
                        nc.scalar.activation(out=e[:, 256:RPC],
                                             in_=sc[:, RPC:RPC + 256],
                                             func=AF.Exp)
                        nc.vector.tensor_mul(out=e, in0=e,
                                             in1=mh_sb[:, s256 - 4, m, :])
                        nc.tensor.matmul(
                            pa[:, 256:RPC], v_cache[:, sm, 2 * pr, :],
                            e[:, 0:256], start=False,
                            stop=(sm == 15), skip_group_check=True)
                        nc.tensor.matmul(
                            pb_[:, 256:RPC], v_cache[:, sm, 2 * pr + 1, :],
                            e[:, 256:RPC], start=False,
                            stop=(sm == 15), skip_group_check=True)
                for head_ps, half in ((pa, 0), (pb_, 1)):
                    inv = pBs.tile([1, RPC], f32, tag="inv")
                    nc.vector.reciprocal(out=inv, in_=head_ps[DH:DH + 1, :])
                    bc = ps_sc.tile([DH, RPC], f32, tag="bc", bufs=1)
                    nc.tensor.matmul(bc, ones64[:, :], inv[:, :],
                                     start=True, stop=True)
                    bcs = pBs.tile([DH, RPC], f32, tag="bcs")
                    nc.scalar.copy(out=bcs, in_=bc)
                    nc.vector.tensor_mul(
                        out=anorm[half * DH:(half + 1) * DH, pr, :],
                        in0=head_ps[0:DH, :], in1=bcs)

        # ========== phase 5: proj + residual + LN2 + ff1 ==========
        with tc.tile_pool(name="ffn", bufs=1) as pC, \
             tc.tile_pool(name="ffns", bufs=3) as pCs, \
             tc.tile_pool(name="ffns2", bufs=2) as pCs2:
          with tc.tile_pool(name="ps_p", bufs=2, space="PSUM") as ps_p:
            for nn in range(2):
                pps = [ps_p.tile([P, 512], f32, tag=f"ppj{tb}", bufs=1,
                                 name=f"ppj{tb}") for tb in range(NTB)]
                for pr in range(NPAIR):
                    wpl = pCs.tile([P, 512], f32r, tag="wpl")
                    nc.sync.dma_start(
                        out=wpl, in_=wp[pr * P:(pr + 1) * P,
                                        nn * 512:(nn + 1) * 512].bitcast(f32r))
                    for tb in range(NTB):
                        nc.tensor.matmul(
                            pps[tb], anorm[:, pr, tb * P:(tb + 1) * P],
                            wpl[:, :], start=(pr == 0), stop=(pr == 7))
                for tb in range(NTB):
                    sl = x_sb[:, tb, nn * 512:(nn + 1) * 512]
                    nc.vector.tensor_add(out=sl, in0=sl, in1=pps[tb])
                    nc.vector.tensor_add(
                        out=sl, in0=sl, in1=bp_bc[:, nn * 512:(nn + 1) * 512])

            h2T = pC.tile([P, 8, RPC], f32r)
            layer_norm_transpose(x_sb, g2_sb, be2_sb, h2T, pCs2, ps_p)

            ffT = pC.tile([P, 32, RPC], bf16)
            for ch in range(4):      # 8 f-blocks per chunk
                w1l = pCs.tile([P, 8, 1024], f32r, tag="w1l", bufs=2)
                nc.sync.dma_start(
                    out=w1l,
                    in_=w1[:, ch * 1024:(ch + 1) * 1024]
                    .rearrange("(cc p) f -> p cc f", p=P).bitcast(f32r))
                for fbl in range(8):
                    fb = ch * 8 + fbl
                    pf = ps_p.tile([P, RPC], f32, tag="pf")
                    for cc in range(8):
                        nc.tensor.matmul(
                            pf, w1l[:, cc, fbl * P:(fbl + 1) * P],
                            h2T[:, cc, :], start=(cc == 0), stop=(cc == 7))
                    nc.scalar.activation(out=ffT[:, fb, :], in_=pf, func=AF.Relu,
                                         bias=b1_sb[:, fb:fb + 1])

          # ---- ff2: single pass, 8 live psum banks ----
          with tc.tile_pool(name="ff2s", bufs=4) as pDs, \
                 tc.tile_pool(name="ff2o", bufs=2) as pDo, \
                 tc.tile_pool(name="ps_o", bufs=1, space="PSUM") as ps_o:
                pos = [ps_o.tile([P, 512], f32, tag=f"po{i}", name=f"po{i}")
                       for i in range(8)]
                for fb in range(32):
                    w2l = pDs.tile([P, C], bf16, tag="w2l")
                    nc.sync.dma_start(out=w2l, in_=w2[fb * P:(fb + 1) * P, :])
                    for tb in range(NTB):
                        for nn in range(2):
                            nc.tensor.matmul(
                                pos[tb * 2 + nn], ffT[:, fb, tb * P:(tb + 1) * P],
                                w2l[:, nn * 512:(nn + 1) * 512],
                                start=(fb == 0), stop=(fb == 31))
                for tb in range(NTB):
                    for nn in range(2):
                        ot = pDo.tile([P, 512], f32, tag="ot")
                        nc.vector.tensor_add(
                            out=ot, in0=pos[tb * 2 + nn],
                            in1=x_sb[:, tb, nn * 512:(nn + 1) * 512])
                        nc.vector.tensor_add(
                            out=ot, in0=ot, in1=b2_bc[:, nn * 512:(nn + 1) * 512])
                        nc.sync.dma_start(
                            out=outr[tb * P:(tb + 1) * P,
                                     nn * 512:(nn + 1) * 512],
                            in_=ot)

    nc.finalize()
    return nc


_NC = None
LAST_RESULTS = None


def _get_nc():
    global _NC
    if _NC is None:
        _NC = build_nc()
    return _NC


def _make_masks(r):
    """Multiplicative causal masks (bf16 1/0) for local rank r."""
    t128 = _t128_list(r)
    tglob = np.concatenate([tb * P + np.arange(P) for tb in t128])  # [512]
    lo = np.zeros((4, 2, P, 2 * RPC), np.float32)
    hi = np.zeros((4, 2, P, RPC), np.float32)
    for s256 in range(4):
        for m in range(2):
            sglob = (2 * s256 + m) * P + np.arange(P)
            keep = (sglob[:, None] <= tglob[None, :]).astype(np.float32)
            lo[s256, m, :, 0:RPC] = keep
            lo[s256, m, :, RPC:2 * RPC] = keep
    for s256 in range(4, 8):
        for m in range(2):
            sglob = (2 * s256 + m) * P + np.arange(P)
            keep = (sglob[:, None] <= tglob[None, 256:512]).astype(np.float32)
            hi[s256 - 4, m, :, 0:256] = keep
            hi[s256 - 4, m, :, 256:RPC] = keep
    return lo.astype(ml_dtypes.bfloat16), hi.astype(ml_dtypes.bfloat16)


def kernel(**inputs):
    x = np.ascontiguousarray(np.asarray(inputs["x"], dtype=np.float32))
    wq = np.asarray(inputs["wq"], dtype=np.float32)   # [H, C, DH]
    wk = np.asarray(inputs["wk"], dtype=np.float32)
    wv = np.asarray(inputs["wv"], dtype=np.float32)

    scale = DH ** -0.5
    wq_flat = np.ascontiguousarray(
        (wq * scale).transpose(1, 0, 2).reshape(C, H * DH).astype(np.float32))
    wk_flat = np.ascontiguousarray(
        wk.transpose(1, 0, 2).reshape(C, H * DH).astype(np.float32))
    wv_flat = np.ascontiguousarray(
        wv.transpose(1, 0, 2).reshape(C, H * DH).astype(np.float32))

    common = {
        "wq": wq_flat,
        "wk": wk_flat,
        "wv": wv_flat,
        "wp": np.ascontiguousarray(np.asarray(inputs["w_proj"], np.float32)),
        "bp": np.ascontiguousarray(np.asarray(inputs["b_proj"], np.float32)),
        "w1": np.ascontiguousarray(np.asarray(inputs["w1"], np.float32)),
        "b1": np.ascontiguousarray(np.asarray(inputs["b1"], np.float32)),
        "w2": np.ascontiguousarray(
            np.asarray(inputs["w2"], np.float32).astype(ml_dtypes.bfloat16)),
        "b2": np.ascontiguousarray(np.asarray(inputs["b2"], np.float32)),
        "g1": np.ascontiguousarray(np.asarray(inputs["g1"], np.float32)),
        "be1": np.ascontiguousarray(np.asarray(inputs["be1"], np.float32)),
        "g2": np.ascontiguousarray(np.asarray(inputs["g2"], np.float32)),
        "be2": np.ascontiguousarray(np.asarray(inputs["be2"], np.float32)),
        "ident": np.eye(P, dtype=np.float32),
        "onesd": np.ones((1, DH), dtype=np.float32),
    }

    in_maps = []
    for c in range(NCORES):
        g, r = c // 4, c % 4
        rows = np.concatenate(
            [x[g, 256 * r:256 * r + 256], x[g, 256 * (7 - r):256 * (7 - r) + 256]])
        m = dict(common)
        m["xr"] = np.ascontiguousarray(rows)
        lo, hi = _make_masks(r)
        m["mask_lo"] = lo
        m["mask_hi"] = hi
        in_maps.append(m)

    nc = _get_nc()
    res = run_bass_kernel_spmd(nc, in_maps, core_ids=list(range(NCORES)))
    global LAST_RESULTS
    LAST_RESULTS = res

    out = np.empty((B, T, C), np.float32)
    for c in range(NCORES):
        g, r = c // 4, c % 4
        o = res.results[c]["outr"]
        out[g, 256 * r:256 * r + 256] = o[0:256]
        out[g, 256 * (7 - r):256 * (7 - r) + 256] = o[256:512]
    return out
